# revision 72
# baseline (speedup 1.0000x reference)
"""Dense 3-layer GAT on 8 TRN2 NeuronCores.

Sharding: each core owns 512 query nodes (rows of the attention score
matrix). Per layer, each core computes h = x @ W for its own nodes,
AllGathers h (bf16), then computes its 512-query slab of masked-softmax
attention and the attended output.

Structure:
- adjacency mask folded into the logits as an additive -30000 logmask;
  score blocks are split between a DVE path (z-add with mask folded in
  + prelu, then ACT exp) and an ACT path (Prelu with per-partition
  bias, paired DVE mask-add, exp) to balance the two engines.
- softmax denominators via an all-ones [128,128] stationary matmul
  accumulating broadcast row-sums in PSUM (L0/L1); for L2 a ones
  column appended to the gathered h makes the row-sum fall out of the
  attended-output matmul itself.
- f = x @ (W@a) computed with WA as the stationary operand (out [8,S]
  directly in f^T layout, bf16); layer-0 f is computed host-side (it
  depends only on the inputs) and shipped in both layouts, so layer 0
  needs no f AllGather.
- layer-0 h gathers are one per head (head 0 split into two
  feature-column halves) so the first attention matmuls start as soon
  as the collective subsystem comes up; head-0's ob2/3 matmul waves
  are emitted one wave behind ob0/1 so a late second half never blocks
  ready work in the in-order PE queue.
- layer boundaries software-pipelined: the next layer's f/h matmuls
  are emitted right after the last head's eviction with the AllGathers
  interleaved (L2's f accumulates incrementally during L1's eviction);
  heads are software-pipelined S0 S1 E0 S2 E1 S3 E2 E3 so evictions
  never stall the score pipeline.
- gathered-h tiles prefetched on the sync DMA queue several key blocks
  ahead; normalization reads attention PSUM directly.
"""

from contextlib import ExitStack

import numpy as np
import ml_dtypes

import concourse.mybir as mybir
import concourse.tile as tile
from concourse import bacc
from concourse.bass_utils import run_bass_kernel_spmd
from concourse.masks import make_identity

P = 128
N_NODES = 4096
S = 512                    # nodes per core
NB = N_NODES // P          # 32 global key blocks
H = 4
J = 2 * H                  # f rows (src/dst per head)
LAYERS = [(512, 512), (2048, 512), (2048, 64)]
F32 = mybir.dt.float32
BF16 = mybir.dt.bfloat16
AF = mybir.ActivationFunctionType
ALU = mybir.AluOpType
NEG = -30000.0
HG_AHEAD = 6               # hg prefetch depth (key blocks)

_CACHE = {}


def _build():
    nc = bacc.Bacc("TRN2", target_bir_lowering=False, debug=False, num_devices=8)

    x0own_d = nc.dram_tensor("x0own", [512, S], BF16, kind="ExternalInput")
    f0T_d = nc.dram_tensor("f0T", [64, S], F32, kind="ExternalInput")
    f0own_d = nc.dram_tensor("f0own", [J, S], BF16, kind="ExternalInput")
    lmask_d = nc.dram_tensor("lmaskT", [P, NB * S], BF16, kind="ExternalInput")
    W_d = []
    WA_d = []
    for li, (fin, fout) in enumerate(LAYERS):
        wshape = [H, fin, fout] if li < 2 else [fin, H * 64]
        W_d.append(nc.dram_tensor(f"W{li}", wshape, BF16, kind="ExternalInput"))
        WA_d.append(nc.dram_tensor(f"WA{li}", [fin, J], BF16, kind="ExternalInput"))
    outT_d = nc.dram_tensor("outT", [H * 64, S], F32, kind="ExternalOutput")

    with tile.TileContext(nc) as tc:
        with ExitStack() as ctx:
            constp = ctx.enter_context(tc.tile_pool(name="const", bufs=1))
            lmp = ctx.enter_context(tc.tile_pool(name="lmp", bufs=1))
            x0p = ctx.enter_context(tc.tile_pool(name="x0p", bufs=8))
            xop = ctx.enter_context(tc.tile_pool(name="xop", bufs=4))
            xbp = ctx.enter_context(tc.tile_pool(name="xbp", bufs=26))
            wtp = ctx.enter_context(tc.tile_pool(name="wt", bufs=21))
            wap = ctx.enter_context(tc.tile_pool(name="wap", bufs=17))
            hsp = ctx.enter_context(tc.tile_pool(name="hsp", bufs=4))
            hgp = ctx.enter_context(tc.tile_pool(name="hgp", bufs=8))
            hgh = ctx.enter_context(tc.tile_pool(name="hgh", bufs=12))
            fp = ctx.enter_context(tc.tile_pool(name="fp", bufs=2))
            fsrcp = ctx.enter_context(tc.tile_pool(name="fsrcp", bufs=5))
            zp = ctx.enter_context(tc.tile_pool(name="zp", bufs=3))
            stp = ctx.enter_context(tc.tile_pool(name="stp", bufs=9))
            rcp = ctx.enter_context(tc.tile_pool(name="rcp", bufs=2))
            evp = ctx.enter_context(tc.tile_pool(name="evp", bufs=2))
            psO = ctx.enter_context(tc.tile_pool(name="psO", bufs=4, space="PSUM"))
            psR = ctx.enter_context(tc.tile_pool(name="psR", bufs=2, space="PSUM"))
            psA = ctx.enter_context(tc.tile_pool(name="psA", bufs=2, space="PSUM"))
            dr = ctx.enter_context(tc.tile_pool(name="dram", bufs=1, space="DRAM"))

            identb = constp.tile([64, 64], F32, tag="identb")
            make_identity(nc, identb[:])
            ones_r = constp.tile([1, P], BF16, tag="ones_r")
            nc.any.memset(ones_r[:], 1.0)
            ones_rf = constp.tile([1, P], F32, tag="ones_rf")
            nc.any.memset(ones_rf[:], 1.0)
            ones128 = constp.tile([P, P], BF16, tag="ones128")
            nc.any.memset(ones128[:], 1.0)

            # layer-0 own x (bf16), 4 fin-chunks [128, 512]
            x0own = []
            for kb in range(4):
                t = xop.tile([P, S], BF16, tag="x0own")
                nc.sync.dma_start(t[:], x0own_d[kb * P:(kb + 1) * P, :])
                x0own.append(t)

            # shared DRAM buffers for the gathers
            # L0: per-head gathers split into two feature-column halves so
            # the first half lands earlier and ob0/1 attention matmuls can
            # start while the second half is still in flight
            agh0_in = {}
            agh0_out = {}
            for c in range(2):
                agh0_in[(0, c)] = dr.tile(
                    [S, 256], BF16, tag=f"ag0i0_{c}", name=f"ag0i0_{c}")
                agh0_out[(0, c)] = dr.tile(
                    [8, S, 256], BF16, tag=f"ag0o0_{c}",
                    name=f"ag0o0_{c}", addr_space="Shared")
            agh0f_in = {}
            agh0f_out = {}
            for h in range(1, H):
                agh0f_in[h] = dr.tile([S, 512], BF16, tag=f"ag0i{h}",
                                      name=f"ag0i{h}")
                agh0f_out[h] = dr.tile([8, S, 512], BF16, tag=f"ag0o{h}",
                                       name=f"ag0o{h}", addr_space="Shared")
            agh1_in = {}
            agh1_out = {}
            for h in range(H):
                agh1_in[h] = dr.tile([S, 512], BF16, tag=f"ag1i{h}",
                                     name=f"ag1i{h}")
                agh1_out[h] = dr.tile([8, S, 512], BF16, tag=f"ag1o{h}",
                                      name=f"ag1o{h}", addr_space="Shared")
            agh_in = {}
            agh_out = {}
            agf_in = {}
            agf_out = {}
            for li in (1, 2):
                fout = LAYERS[li][1]
                for pp in range(2):
                    agh_in[(li, pp)] = dr.tile(
                        [2, S, fout], BF16, tag=f"aghi{li}_{pp}",
                        name=f"aghi{li}_{pp}")
                    agh_out[(li, pp)] = dr.tile(
                        [8, 2, S, fout], BF16, tag=f"agho{li}_{pp}",
                        name=f"agho{li}_{pp}", addr_space="Shared")
                agf_in[li] = dr.tile([J, S], F32, tag=f"agfi{li}",
                                     name=f"agfi{li}")
                agf_out[li] = dr.tile([8, J, S], F32, tag=f"agfo{li}",
                                      name=f"agfo{li}", addr_space="Shared")

            # L2 hg tiles with a preset ones column (col 64)
            hg65 = []
            for i in range(8):
                t = constp.tile([P, 65], BF16, tag=f"hg65_{i}", name=f"hg65_{i}")
                nc.any.memset(t[:, 64:65], 1.0)
                hg65.append(t)

            def load_w_head(li, h):
                """Load one head's W tiles (li < 2), just-in-time."""
                fin, fout = LAYERS[li]
                KB = fin // P
                tiles = []
                for kb in range(KB):
                    t = wtp.tile([P, fout], BF16, tag="wt",
                                 name=f"w{li}_{h}_{kb}")
                    nc.sync.dma_start(
                        t[:], W_d[li][h, kb * P:(kb + 1) * P, :])
                    tiles.append(t)
                return tiles

            def load_w2():
                KB = LAYERS[2][0] // P
                w = []
                for kb in range(KB):
                    t = wtp.tile([P, H * 64], BF16, tag="wt2",
                                 name=f"w2_{kb}")
                    nc.sync.dma_start(t[:], W_d[2][kb * P:(kb + 1) * P, :])
                    w.append(t)
                return w

            def load_wa_tiles(li):
                fin = LAYERS[li][0]
                KB = fin // P
                tiles = []
                for kb in range(KB):
                    t = wap.tile([P, J], BF16, tag="wa", name=f"wa{li}_{kb}")
                    nc.sync.dma_start(t[:], WA_d[li][kb * P:(kb + 1) * P, :])
                    tiles.append(t)
                return tiles

            # ---------------- layer 0 front end ----------------
            def l0_head_mm(h):
                w0h = load_w_head(0, h)
                for b in range(4):
                    ph = psA.tile([P, S], F32, tag="pa", name=f"ph0_{h}_{b}")
                    for kb in range(4):
                        nc.tensor.matmul(
                            ph[:, 0:512],
                            x0own[kb][:, b * P:(b + 1) * P],
                            w0h[kb][:],
                            start=(kb == 0), stop=(kb == 3),
                        )
                    hsb = hsp.tile([P, 512], BF16, tag="hsb")
                    nc.scalar.copy(hsb[:], ph[:, 0:512])
                    if h == 0:
                        for c in range(2):
                            nc.sync.dma_start(
                                agh0_in[(h, c)][b * P:(b + 1) * P, :],
                                hsb[:, c * 256:(c + 1) * 256])
                    else:
                        nc.sync.dma_start(
                            agh0f_in[h][b * P:(b + 1) * P, :], hsb[:])

            def l0_gather(h):
                if h == 0:
                    for c in range(2):
                        nc.gpsimd.collective_compute(
                            "AllGather", ALU.bypass,
                            replica_groups=[list(range(8))],
                            ins=[agh0_in[(h, c)][:].opt()],
                            outs=[agh0_out[(h, c)][:].opt()],
                        )
                else:
                    nc.gpsimd.collective_compute(
                        "AllGather", ALU.bypass,
                        replica_groups=[list(range(8))],
                        ins=[agh0f_in[h][:].opt()],
                        outs=[agh0f_out[h][:].opt()],
                    )

            # layer-0 f computed host-side; load both layouts
            fsb_own0 = fp.tile([J, S], BF16, tag="fsb8", name="fsb_own0")
            nc.sync.dma_start(fsb_own0[:], f0own_d[:])
            fgat0 = fp.tile([64, S], F32, tag="fgat", name="fgat0")
            nc.sync.dma_start(fgat0[:], f0T_d[:])

            # head h-matmuls, each head's gather starts ASAP
            for h in range(H):
                l0_head_mm(h)
                l0_gather(h)

            lmask_res = lmp.tile([P, NB, S], BF16, tag="lmask")
            nc.sync.dma_start(
                lmask_res[:],
                lmask_d[:].rearrange("p (nb n) -> p nb n", nb=NB))

            def lmask_load(li, mbg):
                return lmask_res[:, 2 * mbg:2 * mbg + 2, :]

            def fdst_prep(li, fgat_src):
                """fgat_src: [64, 512] bf16 tile view (r-major, j-minor rows).
                Returns fT_sb [128, 4, 8, J] f32 with
                fT_sb[ml, mh, r, j] = f[j][r*512 + mh*128 + ml]."""
                ptd = psA.tile([P, S], F32, tag="pa", name=f"ptd{li}")
                for mh in range(4):
                    nc.tensor.transpose(
                        ptd[:, mh * 64:(mh + 1) * 64],
                        fgat_src[:, mh * P:(mh + 1) * P],
                        identb[:],
                    )
                fT_sb = fp.tile([P, 4, 8, J], F32, tag="fdst", name=f"fdst{li}")
                nc.scalar.copy(
                    fT_sb[:], ptd[:, 0:256]
                    .rearrange("p (mh rj) -> p mh rj", mh=4)
                    .rearrange("p mh (r j) -> p mh r j", r=8)
                )
                return fT_sb

            def fsrc_bcast(li, fsb8, h):
                fr = fp.tile([1, S], BF16, tag="fr", name=f"fr{li}_{h}")
                nc.sync.dma_start(fr[:], fsb8[2 * h:2 * h + 1, :])
                pb = psA.tile([P, S], F32, tag="pa", name=f"pb{li}_{h}")
                nc.tensor.matmul(pb[:], ones_r[:], fr[:],
                                 start=True, stop=True)
                t = fsrcp.tile([P, S], BF16, tag="fsrcb", name=f"fsrcb{li}_{h}")
                nc.scalar.copy(t[:], pb[:])
                return t

            fT_sb0 = fdst_prep(0, fgat0)
            fsrcb0 = [fsrc_bcast(0, fsb_own0, h) for h in range(H)]

            # ---------------- per-layer attention + next-layer front end ----
            def hg_load(li, h, mb):
                """Issue DMA for gathered h tile of key block mb, head h."""
                r, bsub = mb // 4, mb % 4
                if li == 0:
                    t = hgp.tile([P, 512], BF16, tag="hg", name="hg0f")
                    nc.sync.dma_start(
                        t[:], agh0f_out[h][r, bsub * P:(bsub + 1) * P, :])
                elif li == 1:
                    t = hgp.tile([P, 512], BF16, tag="hg", name=f"hg{li}")
                    nc.sync.dma_start(
                        t[:], agh1_out[h][r, bsub * P:(bsub + 1) * P, :])
                else:
                    t = hg65[(h * NB + mb) % 8]
                    nc.sync.dma_start(
                        t[:, 0:64],
                        agh_out[(2, h // 2)][r, h % 2, bsub * P:(bsub + 1) * P, :])
                return t

            def emit_scores(li, h, fsrcb, fT_sb, mbg, lm):
                """Score pipeline for one mb pair; returns the st tile.
                Blend: ~half the blocks compute z+prelu on DVE (mask folded
                into the add), the rest use ACT Prelu with a paired DVE
                mask-add, to balance the two engines."""
                lrp = zp.tile([P, 2, S], BF16, tag="lr", name=f"lr{li}")
                # per-layer blend: L2 is score-production-bound (tiny po),
                # so it shifts one more pair per 16 onto the ACT path
                if mbg % 16 < (8 if li == 2 else 9):
                    for i in range(2):
                        mb = 2 * mbg + i
                        z = zp.tile([P, S], BF16, tag="z", name=f"z{li}")
                        nc.vector.scalar_tensor_tensor(
                            z[:], fsrcb[:],
                            fT_sb[:, mb % 4, mb // 4, 2 * h + 1:2 * h + 2],
                            lm[:, i, :],
                            ALU.add, ALU.add,
                        )
                        nc.vector.scalar_tensor_tensor(
                            lrp[:, i, :], z[:], 0.2, z[:], ALU.mult, ALU.max,
                        )
                    exp_src = lrp
                else:
                    for i in range(2):
                        mb = 2 * mbg + i
                        nc.scalar.activation(
                            lrp[:, i, :], fsrcb[:], AF.Prelu,
                            bias=fT_sb[:, mb % 4, mb // 4,
                                       2 * h + 1:2 * h + 2],
                            scale=1.0, alpha=0.2,
                        )
                    lrm = zp.tile([P, 2, S], BF16, tag="lrm", name=f"lrm{li}")
                    nc.vector.tensor_tensor(
                        lrm[:], lrp[:], lm[:], ALU.add)
                    exp_src = lrm
                st = stp.tile([P, 2, S], BF16, tag="st", name=f"st{li}")
                nc.scalar.activation(st[:], exp_src[:], AF.Exp, bias=0.0,
                                     scale=1.0)
                return st

            def scores_and_po_l0(h, fsrcb, fT_sb):
                """Layer-0 attention for one head, in waves of 4 mb pairs:
                scores + row-sums first (gather-independent), then ob0/1
                matmuls from the first gathered half, then ob2/3 from the
                second half."""
                po = [
                    psO.tile([P, S], F32, tag="po", name=f"po0_{h}_{ob}")
                    for ob in range(4)
                ]
                prs = psR.tile([P, S], F32, tag="prs", name=f"prs0_{h}")

                def emit_half(wv, c, hgs, sts):
                    # ob pair (2c, 2c+1) matmuls for one wave from half c
                    for k in range(4):
                        for i in range(2):
                            mb = 8 * wv + 2 * k + i
                            hgc = hgs[2 * k + i][c]
                            for sub in range(2):
                                nc.tensor.matmul(
                                    po[2 * c + sub][:, :],
                                    hgc[:, sub * P:(sub + 1) * P],
                                    sts[k][:, i, :],
                                    start=(mb == 0), stop=(mb == NB - 1),
                                )

                pend = None  # (wv, hgs, sts) whose ob2/3 half is deferred
                for wv in range(4):
                    mbgs = range(4 * wv, 4 * wv + 4)
                    lms = [lmask_load(0, mbg) for mbg in mbgs]
                    sts = [emit_scores(0, h, fsrcb, fT_sb, mbg, lms[k])
                           for k, mbg in enumerate(mbgs)]
                    for k, mbg in enumerate(mbgs):
                        for i in range(2):
                            mb = 2 * mbg + i
                            nc.tensor.matmul(
                                prs[:], ones128[:], sts[k][:, i, :],
                                start=(mb == 0), stop=(mb == NB - 1),
                            )
                    hgs = [[] for _ in range(8)]
                    for c in range(2):
                        for k in range(8):
                            mb = 8 * wv + k
                            r, bsub = mb // 4, mb % 4
                            t = hgh.tile([P, 256], BF16, tag=f"hg0_{c}",
                                         name=f"hg0_{c}")
                            nc.sync.dma_start(
                                t[:],
                                agh0_out[(h, c)][r,
                                                 bsub * P:(bsub + 1) * P, :])
                            hgs[k].append(t)
                    emit_half(wv, 0, hgs, sts)
                    # defer this wave's second-half matmuls until after the
                    # next wave's first half, so a late-arriving gather half
                    # never blocks ready work in the in-order PE queue
                    if pend is not None:
                        emit_half(*pend)
                    pend = (wv, 1, hgs, sts)
                if pend is not None:
                    emit_half(pend[0], pend[1], pend[2], pend[3])
                return po, prs

            def scores_and_po(li, h, fsrcb, fT_sb):
                """Emit score pipeline + attention matmuls for one head.
                Returns (po_tiles, psr_or_None)."""
                if li == 0 and h == 0:
                    return scores_and_po_l0(h, fsrcb, fT_sb)
                fout = LAYERS[li][1]
                nob = 4 if fout == 512 else 1
                po = [
                    psO.tile([P, S], F32, tag="po", name=f"po{li}_{h}_{ob}")
                    for ob in range(nob)
                ]
                if li < 2:
                    prs = psR.tile([P, S], F32, tag="prs", name=f"prs{li}_{h}")
                else:
                    prs = None
                # 3 of 4 heads accumulate the softmax denominators on DVE
                # (bf16 pair accumulator) to offload the PE row-sum stream;
                # the final cross-partition reduce stays on PE (2 matmuls)
                dve_rs = False
                accp = None
                pend_acc = None
                hgq = [hg_load(li, h, mb) for mb in range(HG_AHEAD)]
                lmq = [lmask_load(li, g) for g in range(3)]
                for mbg in range(NB // 2):
                    if mbg + 3 < NB // 2:
                        lmq.append(lmask_load(li, mbg + 3))
                    st = emit_scores(li, h, fsrcb, fT_sb, mbg, lmq[mbg])
                    # deferred dual accumulators: the update for pair k is
                    # emitted after pair k+1's score ops so it never blocks
                    # the DVE queue on the exp, and even/odd pairs use
                    # separate accumulators to relax the serial chain
                    if pend_acc is not None:
                        pend_acc()
                        pend_acc = None
                    if dve_rs:
                        par = mbg % 2
                        if accp is None:
                            accp = [None, None]
                        if accp[par] is None:
                            t = rcp.tile([P, 2, S], BF16, tag=f"accp{par}",
                                         name=f"accp{li}_{h}_{par}")
                            accp[par] = t
                            pend_acc = (lambda st=st, t=t:
                                        nc.vector.tensor_copy(t[:], st[:]))
                        else:
                            t = accp[par]
                            pend_acc = (lambda st=st, t=t:
                                        nc.vector.tensor_tensor(
                                            t[:], t[:], st[:], ALU.add))
                    for i in range(2):
                        mb = 2 * mbg + i
                        if mb + HG_AHEAD < NB:
                            hgq.append(hg_load(li, h, mb + HG_AHEAD))
                        hg = hgq[mb]
                        s_t = st[:, i, :]
                        if li < 2:
                            if not dve_rs:
                                nc.tensor.matmul(
                                    prs[:], ones128[:], s_t,
                                    start=(mb == 0), stop=(mb == NB - 1),
                                )
                            for ob in range(nob):
                                nc.tensor.matmul(
                                    po[ob][:, :],
                                    hg[:, ob * P:(ob + 1) * P],
                                    s_t,
                                    start=(mb == 0), stop=(mb == NB - 1),
                                )
                        else:
                            nc.tensor.matmul(
                                po[0][0:65, :], hg[:], s_t,
                                start=(mb == 0), stop=(mb == NB - 1),
                            )
                if pend_acc is not None:
                    pend_acc()
                if dve_rs:
                    for k in range(4):
                        nc.tensor.matmul(
                            prs[:], ones128[:], accp[k % 2][:, k // 2, :],
                            start=(k == 0), stop=(k == 3),
                        )
                return po, prs

            def evict(li, h, po, prs, xb_next):
                """Normalize + ELU for one head; appends bf16 tiles to
                xb_next (L0/L1) or DMAs the final output (L2)."""
                fout = LAYERS[li][1]
                if li < 2:
                    rb = rcp.tile([P, S], F32, tag="rb", name=f"rb{li}_{h}")
                    nc.vector.reciprocal_approx_fast(rb[:], prs[:])
                    for ob in range(4):
                        t0 = evp.tile([P, S], F32, tag="t0")
                        nc.vector.tensor_tensor(t0[:], po[ob][:], rb[:], ALU.mult)
                        em = evp.tile([P, S], F32, tag="em")
                        nc.scalar.activation(em[:], t0[:], AF.Exp, bias=0.0,
                                             scale=1.0)
                        rl = evp.tile([P, S], F32, tag="rl")
                        nc.scalar.activation(rl[:], t0[:], AF.Relu, bias=0.0,
                                             scale=1.0)
                        xbn = xbp.tile([P, S], BF16, tag="xb", name=f"xb{li}")
                        nc.vector.scalar_tensor_tensor(
                            xbn[:], em[:], -1.0, rl[:], ALU.add, ALU.min)
                        xb_next.append(xbn)
                else:
                    # row 64 of po holds the row-sums
                    rs = rcp.tile([1, S], F32, tag="rs2", name=f"rs2_{h}")
                    nc.scalar.copy(rs[:], po[0][64:65, :])
                    rsr = rcp.tile([1, S], F32, tag="rsr2", name=f"rsr2_{h}")
                    nc.vector.reciprocal_approx_fast(rsr[:], rs[:])
                    pbr = psA.tile([P, S], F32, tag="pa", name=f"pbr2_{h}")
                    nc.tensor.matmul(pbr[0:64, :], ones_rf[:, 0:64], rsr[:],
                                     start=True, stop=True)
                    rbs = rcp.tile([64, S], F32, tag="rbs2", name=f"rbs2_{h}")
                    nc.scalar.copy(rbs[:], pbr[0:64, :])
                    t0f = evp.tile([P, S], F32, tag="t0", name="t02")
                    t0 = t0f[0:64, :]
                    nc.vector.tensor_tensor(t0, po[0][0:64, :], rbs[:],
                                            ALU.mult)
                    emf = evp.tile([P, S], F32, tag="em", name="em2")
                    nc.scalar.activation(emf[0:64, :], t0, AF.Exp, bias=0.0,
                                         scale=1.0)
                    rlf = evp.tile([P, S], F32, tag="rl", name="rl2")
                    nc.scalar.activation(rlf[0:64, :], t0, AF.Relu, bias=0.0,
                                         scale=1.0)
                    x1f = evp.tile([P, S], F32, tag="x12", name="x12")
                    nc.vector.scalar_tensor_tensor(
                        x1f[0:64, :], emf[0:64, :], -1.0, rlf[0:64, :],
                        ALU.add, ALU.min)
                    em2 = evp.tile([P, S], F32, tag="em", name="em2b")
                    nc.scalar.activation(em2[0:64, :], x1f[0:64, :], AF.Exp,
                                         bias=0.0, scale=1.0)
                    rl2 = evp.tile([P, S], F32, tag="rl", name="rl2b")
                    nc.scalar.activation(rl2[0:64, :], x1f[0:64, :], AF.Relu,
                                         bias=0.0, scale=1.0)
                    x2 = evp.tile([P, S], F32, tag="x12", name="x2b")
                    nc.vector.scalar_tensor_tensor(
                        x2[0:64, :], em2[0:64, :], -1.0, rl2[0:64, :],
                        ALU.add, ALU.min)
                    nc.sync.dma_start(outT_d[h * 64:(h + 1) * 64, :],
                                      x2[0:64, :])

            def emit_f_gather(li, psf):
                """Copy accumulated f psum out; L1 gets a dedicated
                AllGather, L2 packs its f into the h pair-0 gather buffer
                (rows S..S+63 of head-part 0)."""
                fsb8 = fp.tile([J, S], F32, tag="fsb8f", name=f"fsb8_{li}")
                nc.scalar.copy(fsb8[:], psf[0:J, :])
                fsb8b = fp.tile([J, S], BF16, tag="fsb8", name=f"fsb8b_{li}")
                nc.vector.tensor_copy(fsb8b[:], fsb8[:])
                nc.sync.dma_start(agf_in[li][:], fsb8[:])
                nc.gpsimd.collective_compute(
                    "AllGather", ALU.bypass,
                    replica_groups=[list(range(8))],
                    ins=[agf_in[li][:].opt()], outs=[agf_out[li][:].opt()],
                )
                return fsb8b

            # L2 f accumulated incrementally as L1 eviction frees x chunks
            f2_state = {}

            def f2_partial(xb_cur, upto_kb):
                if "wa" not in f2_state:
                    f2_state["wa"] = load_wa_tiles(2)
                    f2_state["psf"] = psA.tile([P, S], F32, tag="pa",
                                               name="psf2")
                    f2_state["kb"] = 0
                for kb in range(f2_state["kb"], upto_kb):
                    nc.tensor.matmul(
                        f2_state["psf"][0:J, :], f2_state["wa"][kb][:],
                        xb_cur[kb][:],
                        start=(kb == 0), stop=(kb == 15),
                    )
                f2_state["kb"] = upto_kb

            def next_front_end(li, xb_cur):
                """f + h matmuls and gathers for layer li (1 or 2), reading
                xb_cur (16 bf16 [128,512] fin-chunk tiles)."""
                fin, fout = LAYERS[li]
                KB = fin // P
                if li == 1:
                    wa = load_wa_tiles(li)
                    psf = psA.tile([P, S], F32, tag="pa", name=f"psf{li}")
                    for kb in range(KB):
                        nc.tensor.matmul(
                            psf[0:J, :], wa[kb][:], xb_cur[kb][:],
                            start=(kb == 0), stop=(kb == KB - 1),
                        )
                    fsb8b = emit_f_gather(li, psf)
                    # h matmuls with just-in-time weight loads, one gather
                    # per head so the first lands as early as possible;
                    # fdst/fsrc prep is emitted mid-loop so the PE queue
                    # runs straight from the last h matmul into attention
                    prep = {}
                    for h in range(H):
                        wh = load_w_head(li, h)
                        for b in range(4):
                            ph = psA.tile([P, S], F32, tag="pa",
                                          name=f"ph{li}_{h}_{b}")
                            for kb in range(KB):
                                nc.tensor.matmul(
                                    ph[:, 0:fout],
                                    xb_cur[kb][:, b * P:(b + 1) * P],
                                    wh[kb][:],
                                    start=(kb == 0), stop=(kb == KB - 1),
                                )
                            hsb = hsp.tile([P, fout], BF16, tag="hsb")
                            nc.scalar.copy(hsb[:], ph[:, 0:fout])
                            nc.sync.dma_start(
                                agh1_in[h][b * P:(b + 1) * P, :], hsb[:])
                        nc.gpsimd.collective_compute(
                            "AllGather", ALU.bypass,
                            replica_groups=[list(range(8))],
                            ins=[agh1_in[h][:].opt()],
                            outs=[agh1_out[h][:].opt()],
                        )
                        if h == 1:
                            fgat = fp.tile([64, S], F32, tag="fgat",
                                           name=f"fgat{li}")
                            nc.sync.dma_start(
                                fgat[:],
                                agf_out[li][:].rearrange("r j m -> (r j) m"))
                            prep["fT"] = fdst_prep(li, fgat)
                            prep["fs"] = [fsrc_bcast(li, fsb8b, hh)
                                          for hh in range(H)]
                    return prep
                # L2: f was accumulated during L1 attention; gather it first
                f2_partial(xb_cur, 16)
                fsb8b = emit_f_gather(li, f2_state["psf"])
                w = f2_state["w2"]
                # all 4 heads in one 256-wide stream per (b, kb)
                for b in range(4):
                    ph = psA.tile([P, S], F32, tag="pa", name=f"ph2_{b}")
                    for kb in range(KB):
                        nc.tensor.matmul(
                            ph[:, 0:256],
                            xb_cur[kb][:, b * P:(b + 1) * P],
                            w[kb][:],
                            start=(kb == 0), stop=(kb == KB - 1),
                        )
                    hsb = hsp.tile([P, 256], BF16, tag="hsb2")
                    nc.scalar.copy(hsb[:], ph[:, 0:256])
                    for h in range(H):
                        nc.sync.dma_start(
                            agh_in[(2, h // 2)][h % 2,
                                                b * P:(b + 1) * P, :],
                            hsb[:, h * 64:(h + 1) * 64])
                for pp in range(2):
                    nc.gpsimd.collective_compute(
                        "AllGather", ALU.bypass,
                        replica_groups=[list(range(8))],
                        ins=[agh_in[(2, pp)][:].opt()],
                        outs=[agh_out[(2, pp)][:].opt()],
                    )
                return fsb8b

            def layer_attention(li, fsrcb, fT_sb, cbs=None):
                """Software-pipelined heads: S(0) S(1) E(0) S(2) E(1) S(3)
                E(2) E(3); returns xb_next. cbs maps an evicted head index
                to a callback(xb_next) emitted right after that eviction."""
                xb_next = []
                pend = []

                def _evict_one():
                    hh, ppo, pprs = pend.pop(0)
                    evict(li, hh, ppo, pprs, xb_next)
                    if cbs and hh in cbs:
                        cbs[hh](xb_next)

                for h in range(H):
                    po, prs = scores_and_po(li, h, fsrcb[h], fT_sb)
                    pend.append((h, po, prs))
                    if len(pend) >= 2:
                        _evict_one()
                while pend:
                    _evict_one()
                return xb_next

            # ---- run the three layers ----
            xb1 = layer_attention(0, fsrcb0, fT_sb0)

            prep1 = next_front_end(1, xb1)
            f2_state["w2"] = load_w2()
            xb2 = layer_attention(
                1, prep1["fs"], prep1["fT"],
                cbs={1: lambda xb: f2_partial(xb, 8),
                     2: lambda xb: f2_partial(xb, 12),
                     3: lambda xb: f2_partial(xb, 16)})

            fsb8_2 = next_front_end(2, xb2)
            fgat2 = fp.tile([64, S], F32, tag="fgat", name="fgat2")
            nc.sync.dma_start(
                fgat2[:], agf_out[2][:].rearrange("r j m -> (r j) m"))
            fT_sb2 = fdst_prep(2, fgat2)
            fsrcb2 = [fsrc_bcast(2, fsb8_2, h) for h in range(H)]
            layer_attention(2, fsrcb2, fT_sb2)

    nc.compile()
    return nc


def build_in_maps(inputs):
    node_feats = np.asarray(inputs["node_feats"], dtype=np.float32)
    adj = np.asarray(inputs["adj"], dtype=np.float32)
    Ws = [np.asarray(inputs[f"W{i}"], dtype=np.float32) for i in range(3)]
    As = [np.asarray(inputs[f"a{i}"], dtype=np.float32) for i in range(3)]

    WAs = []
    WAs64 = []
    for W, a in zip(Ws, As):
        wa64 = np.einsum(
            "hfo,hjo->fhj", W.astype(np.float64), a.astype(np.float64)
        ).reshape(W.shape[1], J)
        WAs64.append(wa64)
        WAs.append(np.ascontiguousarray(wa64.astype(ml_dtypes.bfloat16)))
    Wbf = [Ws[0].astype(ml_dtypes.bfloat16), Ws[1].astype(ml_dtypes.bfloat16)]
    # L2 weights merged across heads: [fin, H*64]
    W2m = np.ascontiguousarray(
        np.transpose(Ws[2], (1, 0, 2)).reshape(Ws[2].shape[1], H * 64)
    ).astype(ml_dtypes.bfloat16)

    x0T = np.ascontiguousarray(node_feats.T).astype(ml_dtypes.bfloat16)
    # layer-0 f = x @ (W0@a0) for all nodes, in the gather layout
    # f0T[(r*8+j), c] = f0[r*512+c, j]
    f0 = (node_feats.astype(np.float64) @ WAs64[0]).astype(np.float32)  # [N, J]
    f0T = np.ascontiguousarray(
        f0.reshape(8, S, J).transpose(0, 2, 1).reshape(64, S)
    ).astype(np.float32)
    in_maps = []
    for c in range(8):
        rows = slice(c * S, (c + 1) * S)
        lmT = NEG * (1.0 - adj[rows].T)          # [key m, own q]
        lmask = np.ascontiguousarray(
            lmT.reshape(NB, P, S).transpose(1, 0, 2).reshape(P, NB * S)
        ).astype(ml_dtypes.bfloat16)
        m = {
            "x0own": np.ascontiguousarray(x0T[:, rows]),
            "f0T": f0T,
            "f0own": np.ascontiguousarray(f0[rows].T).astype(ml_dtypes.bfloat16),
            "lmaskT": np.ascontiguousarray(lmask),
            "W0": Wbf[0], "W1": Wbf[1], "W2": W2m,
            "WA0": WAs[0], "WA1": WAs[1], "WA2": WAs[2],
        }
        in_maps.append(m)
    return in_maps


def kernel(**inputs):
    if "nc" not in _CACHE:
        _CACHE["nc"] = _build()
    nc = _CACHE["nc"]
    in_maps = build_in_maps(inputs)
    res = run_bass_kernel_spmd(nc, in_maps, core_ids=list(range(8)))
    out = np.concatenate([r["outT"].T for r in res.results], axis=0)
    return np.ascontiguousarray(out, dtype=np.float32)


if __name__ == "__main__":
    rng = np.random.default_rng(0)
    fake = {
        "node_feats": rng.standard_normal((N_NODES, 512), dtype=np.float32),
        "edge_feats": rng.standard_normal((131072, 16), dtype=np.float32),
        "edge_indices": rng.integers(0, N_NODES, (2, 131072)).astype(np.int32),
        "adj": np.maximum(
            (rng.random((N_NODES, N_NODES)) < 0.01).astype(np.float32),
            np.eye(N_NODES, dtype=np.float32),
        ),
    }
    for i, (fin, fout) in enumerate(LAYERS):
        fake[f"W{i}"] = (rng.standard_normal((H, fin, fout)) * 0.05).astype(np.float32)
        fake[f"a{i}"] = (rng.standard_normal((H, 2, fout)) * 0.05).astype(np.float32)
    o = kernel(**fake)
    print("kernel output", o.shape, o.dtype, np.abs(o).mean())
